# revision 20
# baseline (speedup 1.0000x reference)
"""Trainium2 Bass kernel for nn_DeepSC (dual-branch transformer with pairwise Gumbel mask).

Self-contained: hardcodes shapes; shards over 8 NeuronCores as
(batch b = core//4) x (query chunk qc = core%4, 128 queries each).
Per layer each core computes the full per-token prep for its batch (LN, K/V,
fused) plus attention/FFN/mask rows for its own 128 queries; updated chunks are
AllGathered across the 4 cores of each batch group between layers.
"""
import sys
sys.path.insert(0, "/opt/trn_rl_repo")

import numpy as np

import concourse.bass as bass
import concourse.tile as tile
from concourse import bacc, mybir
from concourse.bass_utils import run_bass_kernel_spmd
from concourse.masks import make_identity

F32 = mybir.dt.float32
BF16 = mybir.dt.bfloat16
AF = mybir.ActivationFunctionType
OP = mybir.AluOpType

# Model dims
L, D, H, G, B = 4, 256, 8, 512, 2
HD = D // H            # 32
DFF = 4 * D            # 1024
ALPHA, TAU, EPS, MASK_START = 0.1, 0.3, 1e-8, 2
QN = 128               # queries per core
N_CORES = 8
DC = D // 128          # 2 feature chunks
KC = G // 128          # 4 key chunks
TC = G // 128          # 4 token chunks
HC = DFF // 128        # 8 hidden chunks
RSQ_HD = float(1.0 / np.sqrt(float(HD)))
import os
N_LAYERS = int(os.environ.get("KERNEL_LAYERS", str(L)))
USE_MASK = os.environ.get("KERNEL_MASK", "1") == "1"
GELU_AF = None  # set at build time
STAGE = int(os.environ.get("KERNEL_STAGE", "99"))
NOCC = os.environ.get("KERNEL_NOCC", "0") == "1"

# relu engine split: tile index t mod 16 -> engine
RELU_SPLIT = tuple(int(x) for x in os.environ.get("KERNEL_RELU", "10,4").split(","))  # vector,scalar rest->gpsimd


def _relu_engine(nc, t):
    t = t % 16
    if t < RELU_SPLIT[0]:
        return nc.vector
    if t < RELU_SPLIT[0] + RELU_SPLIT[1]:
        return nc.scalar
    return nc.gpsimd


def build_program():
    nc = bacc.Bacc("TRN2", target_bir_lowering=False, debug=False, num_devices=N_CORES)

    def din(name, shape, dt=F32):
        return nc.dram_tensor(name, shape, dt, kind="ExternalInput").ap()

    ge0 = din("ge0", [G, D])
    ee0 = din("ee0", [G, D])
    geq0 = din("geq0", [QN, D])
    eeq0 = din("eeq0", [QN, D])
    gna = din("gna", [L - MASK_START, 3, KC, 128, QN])
    wd = {}
    for nm in ["wv_g", "wq_g", "wk_g", "wo_g", "wv_e", "wq_e", "wk_e", "wo_e"]:
        wd[nm] = din(nm, [L, D, D])
    wfuse = din("wfuse", [L, 2 * D, D])
    wf1 = {"g": din("wf1_g", [L, D, DFF], BF16), "e": din("wf1_e", [L, D, DFF], BF16)}
    wf2 = {"g": din("wf2_g", [L, DFF, D], BF16), "e": din("wf2_e", [L, DFF, D], BF16)}
    bd = {}
    for nm in ["bv_g", "bq_g", "bk_g", "bo_g", "bv_e", "bq_e", "bk_e", "bo_e", "bfuse",
               "bf2_g", "bf2_e"]:
        bd[nm] = din(nm, [L, D])
    bf1 = {"g": din("bf1_g", [L, DFF]), "e": din("bf1_e", [L, DFF])}
    lnp = {}
    for nm in ["ln_g1", "ln_g2", "ln_e1", "ln_e2"]:
        lnp[nm + "_w"] = din(nm + "_w", [L, D])
        lnp[nm + "_b"] = din(nm + "_b", [L, D])
    wg1s = din("wg1s", [D, D])
    wg1t = din("wg1t", [D, D])
    bg1d = din("bg1", [D])
    w2pad = din("w2pad", [D, 32], BF16)

    out = nc.dram_tensor("out", [2, QN, D], F32, kind="ExternalOutput").ap()

    import contextlib
    with tile.TileContext(nc) as tc, contextlib.ExitStack() as ctx:
        sb = ctx.enter_context(tc.tile_pool(name="sb", bufs=1))
        sb2 = ctx.enter_context(tc.tile_pool(name="sb2", bufs=2))
        sbw = ctx.enter_context(tc.tile_pool(name="sbw", bufs=1))
        sb1 = ctx.enter_context(tc.tile_pool(name="sb1", bufs=1))
        sbr = ctx.enter_context(tc.tile_pool(name="sbr", bufs=int(os.environ.get("SBR","6"))))
        sb4 = ctx.enter_context(tc.tile_pool(name="sb4", bufs=4))
        ps = ctx.enter_context(tc.tile_pool(name="ps", bufs=int(os.environ.get("PS_MM","4")), space="PSUM"))
        psa = ctx.enter_context(tc.tile_pool(name="psa", bufs=1, space="PSUM"))
        psb = ctx.enter_context(tc.tile_pool(name="psb", bufs=int(os.environ.get("PS_MB","2")), space="PSUM"))
        pst = ctx.enter_context(tc.tile_pool(name="pst", bufs=1, space="PSUM"))
        dram = ctx.enter_context(tc.tile_pool(name="dram", bufs=2, space="DRAM"))

        ident = sb1.tile([128, 128], F32, tag="ident")
        make_identity(nc, ident)
        ones_bf = sb1.tile([128, 1], BF16, tag="ones_bf")
        nc.vector.memset(ones_bf, 1.0)
        eps_ln = sb1.tile([128, 1], F32, tag="eps_ln")
        nc.vector.memset(eps_ln, 1e-5)

        w1s_sb = sb1.tile([128, DC, D], F32, tag="w1s_sb")
        nc.sync.dma_start(out=w1s_sb, in_=wg1s.rearrange("(c p) m -> p c m", c=DC))
        w1t_sb = sb1.tile([128, DC, D], F32, tag="w1t_sb")
        nc.sync.dma_start(out=w1t_sb, in_=wg1t.rearrange("(c p) m -> p c m", c=DC))
        bg1_sb = sb1.tile([128, DC], F32, tag="bg1_sb")
        nc.sync.dma_start(out=bg1_sb, in_=bg1d.rearrange("(c p) -> p c", c=DC))
        w2_sb = sb1.tile([128, DC, 32], BF16, tag="w2_sb")
        nc.sync.dma_start(out=w2_sb, in_=w2pad.rearrange("(c p) m -> p c m", c=DC))

        def load_full(dr, tag):
            ts_ = []
            for t in range(TC):
                x = sb.tile([128, D], F32, tag=f"{tag}{t}")
                nc.sync.dma_start(out=x, in_=dr[128 * t:128 * (t + 1), :])
                ts_.append(x)
            return ts_

        ge = load_full(ge0, "ge")
        ee = load_full(ee0, "ee")
        geq = sb.tile([128, D], F32, tag="geq")
        nc.sync.dma_start(out=geq, in_=geq0)
        eeq = sb.tile([128, D], F32, tag="eeq")
        nc.sync.dma_start(out=eeq, in_=eeq0)

        def transpose_full(xts, tag, wvec=None, bvec=None):
            outs = []
            for dcc in range(DC):
                p = ps.tile([128, 128 * len(xts)], F32, tag="mm")
                for t, x in enumerate(xts):
                    nc.tensor.transpose(p[:, 128 * t:128 * (t + 1)],
                                        x[:, 128 * dcc:128 * (dcc + 1)], ident)
                o = sb.tile([128, 128 * len(xts)], F32, tag=f"{tag}{dcc}")
                if wvec is not None:
                    nc.vector.tensor_scalar(out=o, in0=p, scalar1=wvec[:, dcc:dcc + 1],
                                            scalar2=bvec[:, dcc:dcc + 1],
                                            op0=OP.mult, op1=OP.add)
                else:
                    nc.scalar.activation(out=o, in_=p, func=AF.Copy)
                outs.append(o)
            return outs

        def layer_norm(xts, w_dr, b_dr, li, tag):
            wv = sb.tile([128, DC], F32, tag=f"lnw_{tag}")
            nc.sync.dma_start(out=wv, in_=w_dr[li].rearrange("(c p) -> p c", c=DC))
            bv = sb.tile([128, DC], F32, tag=f"lnb_{tag}")
            nc.sync.dma_start(out=bv, in_=b_dr[li].rearrange("(c p) -> p c", c=DC))
            normed = []
            for t, x in enumerate(xts):
                st = sb.tile([128, 6], F32, tag=f"lnst_{tag}{t}")
                nc.vector.bn_stats(out=st, in_=x)
                mv = sb.tile([128, 2], F32, tag=f"lnmv_{tag}{t}")
                nc.vector.bn_aggr(out=mv, in_=st)
                rstd = sb.tile([128, 1], F32, tag=f"lnrs_{tag}{t}")
                nc.scalar.activation(out=rstd, in_=mv[:, 1:2], func=AF.Sqrt,
                                     bias=eps_ln, scale=1.0)
                nc.vector.reciprocal(out=rstd, in_=rstd)
                xn = sb.tile([128, D], F32, tag=f"lnxn_{tag}{t}")
                nc.vector.tensor_scalar(out=xn, in0=x, scalar1=mv[:, 0:1],
                                        scalar2=rstd, op0=OP.subtract, op1=OP.mult)
                normed.append(xn)
            return transpose_full(normed, f"xh_{tag}", wv, bv), normed

        def proj_head_split(xT, w_dr, li, b_dr, tag, ws=None):
            """K/Q projection restaged per-head at partition base 0:
            returns [32, H, N] bf16 tile (head h = cols [h*N:(h+1)*N])."""
            N = xT[0].shape[-1]
            bv = sb.tile([128, DC], F32, tag=f"pb_{tag}")
            nc.sync.dma_start(out=bv, in_=b_dr[li].rearrange("(c p) -> p c", c=DC))
            if ws is None:
                ws = sbw.tile([128, DC, D], F32, tag=f"pw_{tag}")
                nc.sync.dma_start(out=ws,
                                  in_=w_dr[li].rearrange("(c p) m -> p c m", c=DC))
            khq = sb.tile([32, H, N], BF16, tag=f"hs_{tag}")
            for mc in range(DC):
                p = ps.tile([128, N], F32, tag="mm")
                for kcc in range(DC):
                    nc.tensor.matmul(p, ws[:, kcc, 128 * mc:128 * (mc + 1)], xT[kcc],
                                     start=(kcc == 0), stop=(kcc == DC - 1))
                for hl in range(4):
                    h = mc * 4 + hl
                    if hl % 2 == 0:
                        nc.vector.tensor_scalar(
                            out=khq[0:32, h, :], in0=p[32 * hl:32 * hl + 32, :],
                            scalar1=bv[32 * hl:32 * hl + 32, mc:mc + 1], scalar2=None,
                            op0=OP.add)
                    else:
                        nc.scalar.activation(
                            out=khq[0:32, h, :], in_=p[32 * hl:32 * hl + 32, :],
                            func=AF.Identity,
                            bias=bv[32 * hl:32 * hl + 32, mc:mc + 1], scale=1.0)
            return khq

        def proj_feature(xT, w_dr, li, b_dr, tag, n_in=DC, scale=None, ws=None):
            """out^T[d_out, N] = W^T @ x^T (+ per-partition bias, optional scale)."""
            N = xT[0].shape[-1]
            bv = None
            if b_dr is not None:
                bv = sb.tile([128, DC], F32, tag=f"pb_{tag}")
                nc.sync.dma_start(out=bv, in_=b_dr[li].rearrange("(c p) -> p c", c=DC))
            if ws is None:
                ws = sbw.tile([128, n_in, D], F32, tag=f"pw_{tag}")
                nc.sync.dma_start(out=ws,
                                  in_=w_dr[li].rearrange("(c p) m -> p c m", c=n_in))
            outs = []
            for mc in range(DC):
                p = ps.tile([128, N], F32, tag="mm")
                for kcc in range(n_in):
                    nc.tensor.matmul(p, ws[:, kcc, 128 * mc:128 * (mc + 1)], xT[kcc],
                                     start=(kcc == 0), stop=(kcc == n_in - 1))
                o = sb.tile([128, N], F32, tag=f"po_{tag}{mc}")
                if bv is not None:
                    nc.vector.tensor_scalar(
                        out=o, in0=p, scalar1=bv[:, mc:mc + 1],
                        scalar2=scale, op0=OP.add,
                        op1=(OP.mult if scale is not None else OP.bypass))
                else:
                    nc.scalar.activation(out=o, in_=p, func=AF.Copy)
                outs.append(o)
            return outs

        def v_tokens(xhT, w_dr, b_dr, li, tag):
            ws = sbw.tile([128, DC, D], F32, tag=f"vw_{tag}")
            nc.sync.dma_start(out=ws, in_=w_dr[li].rearrange("(c p) m -> p c m", c=DC))
            bb = sb.tile([128, D], F32, tag=f"vbb_{tag}")
            nc.sync.dma_start(out=bb, in_=bass.AP(
                tensor=b_dr.tensor, offset=b_dr[li].offset, ap=[[0, 128], [1, D]]))
            outs = []
            for t in range(TC):
                p = ps.tile([128, D], F32, tag="mm")
                for kcc in range(DC):
                    nc.tensor.matmul(p, xhT[kcc][:, 128 * t:128 * (t + 1)], ws[:, kcc],
                                     start=(kcc == 0), stop=(kcc == DC - 1))
                o = sb.tile([128, H, 33], BF16, tag=f"vo_{tag}{t}")
                nc.vector.tensor_tensor(
                    out=o[:, :, 0:32],
                    in0=p.rearrange("p (h m) -> p h m", m=32),
                    in1=bb.rearrange("p (h m) -> p h m", m=32), op=OP.add)
                nc.vector.memset(o[:, :, 32:33], 1.0)
                outs.append(o)
            return outs

        def attn_scores(kT, qT, tag, mT=None):
            As = []
            for kcc in range(KC):
                a = sb2.tile([128, H * 128], BF16, tag=f"as{kcc}_{tag}")
                for hg in range(2):
                    p = ps.tile([128, 512], F32, tag="mm")
                    for hl in range(4):
                        h = hg * 4 + hl
                        nc.tensor.matmul(
                            p[:, 128 * hl:128 * (hl + 1)],
                            kT[0:32, h, 128 * kcc:128 * (kcc + 1)],
                            qT[0:32, h, :],
                            start=True, stop=True)
                    nc.scalar.activation(out=a[:, 512 * hg:512 * (hg + 1)], in_=p,
                                         func=AF.Exp)
                if mT is not None:
                    nc.vector.tensor_tensor(
                        out=a.rearrange("p (h q) -> p h q", h=H),
                        in0=a.rearrange("p (h q) -> p h q", h=H),
                        in1=bass.AP(tensor=mT.tensor,
                                    offset=mT[:, kcc, :].offset,
                                    ap=[mT.ap[0], [0, H], [1, QN]]),
                        op=OP.mult)
                As.append(a)
            return As

        def attn_out(As, v_sb, wo_nm, bo_nm, li, tag, masked):
            op_ = psa.tile([128, H * 33 + H], F32, tag="av")
            for h in range(H):
                for kcc in range(KC):
                    nc.tensor.matmul(op_[:, 33 * h:33 * (h + 1)],
                                     As[kcc][:, 128 * h:128 * (h + 1)],
                                     v_sb[kcc][:, h, :],
                                     start=(kcc == 0), stop=(kcc == KC - 1))
            if masked:
                us = []
                for kcc in range(KC):
                    u = sb4.tile([128, H * 128], BF16, tag="u")
                    nc.vector.tensor_scalar(
                        out=u.bitcast(mybir.dt.uint32), in0=As[kcc].bitcast(mybir.dt.uint32),
                        scalar1=0x7FFF7FFF, scalar2=None, op0=OP.bitwise_and)
                    us.append(u)
                for h in range(H):
                    for kcc in range(KC):
                        nc.tensor.matmul(op_[:, H * 33 + h:H * 33 + h + 1],
                                         us[kcc][:, 128 * h:128 * (h + 1)], ones_bf,
                                         start=(kcc == 0), stop=(kcc == KC - 1))
            dn = sb.tile([128, H], F32, tag=f"dn_{tag}")
            if not masked:
                nc.vector.tensor_copy(dn, op_[:, 32:H * 33:33])
            else:
                nc.vector.tensor_scalar(out=dn, in0=op_[:, H * 33:H * 33 + H],
                                        scalar1=EPS, scalar2=None, op0=OP.add)
            rz = sb.tile([128, H], F32, tag=f"rz_{tag}")
            nc.vector.reciprocal(out=rz, in_=dn)
            o_sb = sb.tile([128, D], F32, tag=f"osb_{tag}")
            nc.vector.tensor_tensor(
                out=o_sb.rearrange("p (h m) -> p h m", h=H),
                in0=op_[:, 0:H * 33].rearrange("p (h m) -> p h m", m=33)[:, :, 0:32],
                in1=bass.AP(tensor=rz.tensor, offset=rz.offset,
                            ap=[rz.ap[0], [1, H], [0, 32]]),
                op=OP.mult)
            oT_p = ps.tile([128, D], F32, tag="mm")
            for dcc in range(DC):
                nc.tensor.transpose(oT_p[:, 128 * dcc:128 * (dcc + 1)],
                                    o_sb[:, 128 * dcc:128 * (dcc + 1)], ident)
            oT = sb.tile([128, D], F32, tag=f"ot_{tag}")
            nc.scalar.activation(out=oT, in_=oT_p, func=AF.Copy)
            attnT = proj_feature([oT[:, 0:128], oT[:, 128:256]], wd[wo_nm], li,
                                 bd[bo_nm], f"o_{tag}")
            at_p = ps.tile([128, D], F32, tag="mm")
            for dcc in range(DC):
                nc.tensor.transpose(at_p[:, 128 * dcc:128 * (dcc + 1)],
                                    attnT[dcc], ident)
            at_sb = sb.tile([128, D], F32, tag=f"at_{tag}")
            nc.scalar.activation(out=at_sb, in_=at_p, func=AF.Copy)
            return at_sb

        def ffn_a(x2q, br, li, tag):
            xhT, xln = layer_norm([x2q], lnp[f"ln_{br}2_w"], lnp[f"ln_{br}2_b"], li,
                                  f"2{tag}")
            xhTb = sb.tile([128, DC, 128], BF16, tag=f"xhb_{tag}")
            for dcc in range(DC):
                nc.scalar.activation(out=xhTb[:, dcc], in_=xhT[dcc], func=AF.Copy)
            w1 = sbw.tile([128, DC, DFF], BF16, tag=f"w1_{tag}")
            nc.sync.dma_start(out=w1, in_=wf1[br][li].rearrange("(c p) m -> p c m", c=DC))
            b1 = sb.tile([128, HC], F32, tag=f"b1_{tag}")
            nc.sync.dma_start(out=b1, in_=bf1[br][li].rearrange("(c p) -> p c", c=HC))
            h1 = sb.tile([128, HC, 128], BF16, tag=f"h1_{tag}")
            for bank in range(2):
                p = ps.tile([128, 512], F32, tag="mm")
                for mc4 in range(4):
                    mc = bank * 4 + mc4
                    for kcc in range(DC):
                        nc.tensor.matmul(p[:, 128 * mc4:128 * (mc4 + 1)],
                                         w1[:, kcc, 128 * mc:128 * (mc + 1)],
                                         xhTb[:, kcc],
                                         start=(kcc == 0), stop=(kcc == DC - 1))
                for mc4 in range(4):
                    mc = bank * 4 + mc4
                    nc.scalar.activation(out=h1[:, mc],
                                         in_=p[:, 128 * mc4:128 * (mc4 + 1)],
                                         func=(AF.Tanh if os.environ.get("KERNEL_SIMGELU") == "1"
                                               else AF.Gelu),
                                         bias=b1[:, mc:mc + 1], scale=1.0)
            return xln[0], h1

        def ffn_b(h1, br, li, tag):
            w2 = sbw.tile([128, HC, D], BF16, tag=f"w2f_{tag}")
            nc.sync.dma_start(out=w2, in_=wf2[br][li].rearrange("(c p) m -> p c m", c=HC))
            b2 = sb.tile([128, DC], F32, tag=f"b2_{tag}")
            nc.sync.dma_start(out=b2, in_=bd[f"bf2_{br}"][li].rearrange("(c p) -> p c",
                                                                        c=DC))
            p = ps.tile([128, D], F32, tag="mm")
            for mc in range(DC):
                for kcc in range(HC):
                    nc.tensor.matmul(p[:, 128 * mc:128 * (mc + 1)],
                                     w2[:, kcc, 128 * mc:128 * (mc + 1)], h1[:, kcc],
                                     start=(kcc == 0), stop=(kcc == HC - 1))
            foT = sb.tile([128, D], F32, tag=f"fo_{tag}")
            for mc in range(DC):
                nc.vector.tensor_scalar(out=foT[:, 128 * mc:128 * (mc + 1)],
                                        in0=p[:, 128 * mc:128 * (mc + 1)],
                                        scalar1=b2[:, mc:mc + 1], scalar2=None,
                                        op0=OP.add)
            fo_p = ps.tile([128, D], F32, tag="mm")
            for dcc in range(DC):
                nc.tensor.transpose(fo_p[:, 128 * dcc:128 * (dcc + 1)],
                                    foT[:, 128 * dcc:128 * (dcc + 1)], ident)
            fo_sb = sb.tile([128, D], F32, tag=f"fos_{tag}")
            nc.scalar.activation(out=fo_sb, in_=fo_p, func=AF.Copy)
            return fo_sb

        def gumbel_mask(geT, geqT, li):
            """Returns mT tile [128 j, KC, QN] bf16 with M^T in {-1,0,+1}."""
            lm = li - MASK_START
            tgtT = []
            for mc in range(DC):
                p = ps.tile([128, G], F32, tag="mm")
                for kcc in range(DC):
                    nc.tensor.matmul(p, w1t_sb[:, kcc, 128 * mc:128 * (mc + 1)],
                                     geT[kcc], start=(kcc == 0), stop=(kcc == DC - 1))
                o = sb.tile([128, G], BF16, tag=f"tgtT{mc}")
                nc.scalar.activation(out=o, in_=p, func=AF.Copy)
                tgtT.append(o)
            srcb = []
            for mc in range(DC):
                p = ps.tile([128, QN], F32, tag="mm")
                for kcc in range(DC):
                    nc.tensor.matmul(p, w1s_sb[:, kcc, 128 * mc:128 * (mc + 1)],
                                     geqT[kcc], start=(kcc == 0), stop=(kcc == DC - 1))
                o = sb.tile([128, QN], F32, tag=f"srcb{mc}")
                nc.vector.tensor_scalar(out=o, in0=p, scalar1=bg1_sb[:, mc:mc + 1],
                                        scalar2=None, op0=OP.add)
                srcb.append(o)
            a_sb = sb.tile([128, 3, KC, QN], F32, tag="gna_sb")
            nc.sync.dma_start(out=a_sb, in_=gna[lm].rearrange("a k p q -> p a k q"))

            uT = sb.tile([128, KC, QN], F32, tag="uT")
            vT = sb.tile([128, KC, QN], F32, tag="vT")
            bank = None
            for i in range(QN):
                r = 32 * (i % 4)
                if i % 4 == 0:
                    bank = psb.tile([128, 512], F32, tag="mbank")
                for dcc in range(DC):
                    eng = _relu_engine(nc, i * 2 + dcc)
                    hh = sbr.tile([128, G], BF16, tag="hh")
                    if eng is nc.scalar:
                        nc.scalar.activation(out=hh, in_=tgtT[dcc], func=AF.Relu,
                                             bias=srcb[dcc][:, i:i + 1], scale=1.0)
                    else:
                        eng.tensor_scalar(out=hh, in0=tgtT[dcc],
                                          scalar1=srcb[dcc][:, i:i + 1], scalar2=0.0,
                                          op0=OP.add, op1=OP.max)
                    nc.tensor.matmul(bank[r:r + 32, :], w2_sb[:, dcc], hh,
                                     start=(dcc == 0), stop=(dcc == DC - 1),
                                     tile_position=(0, r))
                if i % 4 == 3:
                    g4 = i // 4
                    stg = sb.tile([128, 512], F32, tag=f"mstg{g4 % 2}")
                    if g4 % 2 == 0:
                        nc.scalar.activation(out=stg, in_=bank, func=AF.Copy)
                    else:
                        nc.vector.tensor_copy(stg, bank)
                    mtp = pst.tile([128, 512], F32, tag="mtp")
                    for kcc in range(KC):
                        nc.tensor.transpose(mtp[:, 128 * kcc:128 * (kcc + 1)],
                                            stg[:, 128 * kcc:128 * (kcc + 1)], ident)
                    # gather u,v for these 4 queries across all KC chunks
                    src = mtp.rearrange("p (k q s) -> p k q s", k=KC, q=4)
                    eng2 = nc.vector if g4 % 2 == 0 else nc.scalar
                    uo = uT.rearrange("p k q -> p k q")[:, :, 4 * g4:4 * g4 + 4]
                    vo = vT.rearrange("p k q -> p k q")[:, :, 4 * g4:4 * g4 + 4]
                    if eng2 is nc.vector:
                        nc.vector.tensor_copy(out=uo, in_=src[:, :, :, 0])
                        nc.vector.tensor_copy(out=vo, in_=src[:, :, :, 1])
                    else:
                        nc.scalar.activation(out=uo, in_=src[:, :, :, 0], func=AF.Copy)
                        nc.scalar.activation(out=vo, in_=src[:, :, :, 1], func=AF.Copy)

            mT = sb.tile([128, KC, QN], BF16, tag="mT")
            for kcc in range(KC):
                a0 = a_sb[:, 0, kcc]
                a1 = a_sb[:, 1, kcc]
                a2 = a_sb[:, 2, kcc]
                u_ = uT[:, kcc]
                v_ = vT[:, kcc]
                p1 = sb.tile([128, QN], F32, tag="m_p1")
                nc.vector.tensor_tensor(out=p1, in0=v_, in1=a1, op=OP.is_gt)
                vmu = sb.tile([128, QN], F32, tag="m_vmu")
                nc.vector.tensor_tensor(out=vmu, in0=v_, in1=u_, op=OP.subtract)
                p2 = sb.tile([128, QN], F32, tag="m_p2")
                nc.vector.tensor_tensor(out=p2, in0=vmu, in1=a2, op=OP.is_gt)
                plus = sb.tile([128, QN], F32, tag="m_plus")
                nc.vector.tensor_tensor(out=plus, in0=p1, in1=p2, op=OP.mult)
                m1 = sb.tile([128, QN], F32, tag="m_m1")
                nc.vector.tensor_tensor(out=m1, in0=u_, in1=a0, op=OP.is_le)
                notp1 = sb.tile([128, QN], F32, tag="m_np1")
                nc.vector.tensor_scalar(out=notp1, in0=p1, scalar1=-1.0, scalar2=1.0,
                                        op0=OP.mult, op1=OP.add)
                minus = sb.tile([128, QN], F32, tag="m_minus")
                nc.vector.tensor_tensor(out=minus, in0=m1, in1=notp1, op=OP.mult)
                nc.vector.tensor_tensor(out=mT[:, kcc], in0=plus, in1=minus,
                                        op=OP.subtract)
            return mT

        # ---------------- layer loop ----------------
        for li in range(N_LAYERS):
            masked = (li >= MASK_START) and USE_MASK

            geT = transpose_full(ge, "geT")
            eeT = transpose_full(ee, "eeT")
            qTs = {}
            for (src, dst_tag) in [(geq, "geqT"), (eeq, "eeqT")]:
                p = ps.tile([128, D], F32, tag="mm")
                for dcc in range(DC):
                    nc.tensor.transpose(p[:, 128 * dcc:128 * (dcc + 1)],
                                        src[:, 128 * dcc:128 * (dcc + 1)], ident)
                o = sb.tile([128, D], F32, tag=dst_tag)
                nc.scalar.activation(out=o, in_=p, func=AF.Copy)
                qTs[dst_tag] = [o[:, 0:128], o[:, 128:256]]
            geqT, eeqT = qTs["geqT"], qTs["eeqT"]

            mT = gumbel_mask(geT, geqT, li) if masked else None
            if STAGE == 0:
                nc.sync.dma_start(out=out[0], in_=geq)
                nc.sync.dma_start(out=out[1], in_=eeq)
                break

            # ---- interleaved branches ----
            xhgT, _ = layer_norm(ge, lnp["ln_g1_w"], lnp["ln_g1_b"], li, "g1")
            xheT, _ = layer_norm(ee, lnp["ln_e1_w"], lnp["ln_e1_b"], li, "e1")
            xhgqT, _ = layer_norm([geq], lnp["ln_g1_w"], lnp["ln_g1_b"], li, "g1q")
            wfuse_sb = sbw.tile([128, 2 * DC, D], F32, tag="wfuse_sb")
            nc.sync.dma_start(out=wfuse_sb,
                              in_=wfuse[li].rearrange("(c p) m -> p c m", c=2 * DC))
            fusedT = proj_feature(geT + eeT, wfuse, li, bd["bfuse"], "fu",
                                  n_in=2 * DC, ws=wfuse_sb)
            fusedqT = proj_feature(geqT + eeqT, wfuse, li, bd["bfuse"], "fuq",
                                   n_in=2 * DC, ws=wfuse_sb)
            qgT = proj_head_split(xhgqT, wd["wq_g"], li, bd["bq_g"], "qg")
            qeT = proj_head_split(fusedqT, wd["wq_e"], li, bd["bq_e"], "qe")
            kgT = proj_head_split(xhgT, wd["wk_g"], li, bd["bk_g"], "kg")
            keT = proj_head_split(fusedT, wd["wk_e"], li, bd["bk_e"], "ke")
            vg = v_tokens(xhgT, wd["wv_g"], bd["bv_g"], li, "vg")
            ve = v_tokens(xheT, wd["wv_e"], bd["bv_e"], li, "ve")
            As_g = attn_scores(kgT, qgT, "g", mT=mT)
            As_e = attn_scores(keT, qeT, "e", mT=mT)
            attn_g = attn_out(As_g, vg, "wo_g", "bo_g", li, "g", masked)
            attn_e = attn_out(As_e, ve, "wo_e", "bo_e", li, "e", masked)
            x2g = sb.tile([128, D], F32, tag="x2g")
            nc.vector.tensor_tensor(out=x2g, in0=geq, in1=attn_g, op=OP.add)
            x2e = sb.tile([128, D], F32, tag="x2e")
            nc.vector.tensor_tensor(out=x2e, in0=eeq, in1=attn_e, op=OP.add)
            xln_g, h1g = ffn_a(x2g, "g", li, "g")
            xln_e, h1e = ffn_a(x2e, "e", li, "e")
            ffn_g = ffn_b(h1g, "g", li, "g")
            ffn_e = ffn_b(h1e, "e", li, "e")
            ge_new = sb2.tile([128, D], F32, tag="ge_new")
            nc.vector.tensor_tensor(out=ge_new, in0=x2g, in1=xln_g, op=OP.add)
            nc.vector.tensor_tensor(out=ge_new, in0=ge_new, in1=ffn_g, op=OP.add)
            if li < N_LAYERS - 1:
                cin_g = dram.tile([QN, D], F32, tag="cin_g")
                cout_g = dram.tile([G, D], F32, tag="cout_g")
                nc.sync.dma_start(out=cin_g, in_=ge_new)
                if NOCC:
                    for _t in range(TC):
                        nc.sync.dma_start(out=cout_g[128 * _t:128 * (_t + 1), :],
                                          in_=cin_g)
                else:
                    nc.gpsimd.collective_compute(
                        "AllGather", OP.bypass,
                        replica_groups=[[0, 1, 2, 3], [4, 5, 6, 7]],
                        ins=[cin_g.opt()], outs=[cout_g.opt()])
            ee_new = sb2.tile([128, D], F32, tag="ee_new")
            nc.vector.tensor_tensor(out=ee_new, in0=x2e, in1=xln_e, op=OP.add)
            nc.vector.tensor_tensor(out=ee_new, in0=ee_new, in1=ffn_e, op=OP.add)

            if li < N_LAYERS - 1:
                cin_e = dram.tile([QN, D], F32, tag="cin_e")
                cout_e = dram.tile([G, D], F32, tag="cout_e")
                nc.sync.dma_start(out=cin_e, in_=ee_new)
                if NOCC:
                    for _t in range(TC):
                        nc.sync.dma_start(out=cout_e[128 * _t:128 * (_t + 1), :],
                                          in_=cin_e)
                else:
                    nc.gpsimd.collective_compute(
                        "AllGather", OP.bypass,
                        replica_groups=[[0, 1, 2, 3], [4, 5, 6, 7]],
                        ins=[cin_e.opt()], outs=[cout_e.opt()])
                ge = load_full(cout_g, "ge")
                ee = load_full(cout_e, "ee")
                geq, eeq = ge_new, ee_new
            else:
                nc.sync.dma_start(out=out[0], in_=ge_new)
                nc.sync.dma_start(out=out[1], in_=ee_new)

    nc.compile()
    return nc


_NC = None


def _get_nc():
    global _NC
    if _NC is None:
        _NC = build_program()
    return _NC


def kernel(gene_ids, discrete_expression, normalized_expr, gumbel_noise, params):
    import ml_dtypes
    p = {k: np.asarray(v) for k, v in params.items()}
    gene_ids = np.asarray(gene_ids)
    discrete_expression = np.asarray(discrete_expression)
    normalized_expr = np.asarray(normalized_expr, np.float32)
    u = np.asarray(gumbel_noise, np.float32)

    ge = np.asarray(p["gene_table"], np.float32)[gene_ids]
    ee = np.asarray(p["bin_table"], np.float32)[discrete_expression] + \
        ALPHA * normalized_expr[..., None] * np.asarray(p["cont_proj"], np.float32)

    gn = -np.log(-np.log(u + 1e-20) + 1e-20)
    bg2 = np.asarray(p["bg2"], np.float32)
    b10 = float(bg2[1] - bg2[0])
    b20 = float(bg2[2] - bg2[0])
    A0 = gn[..., 0] - gn[..., 1] - b10
    A1 = gn[..., 0] - gn[..., 2] - b20
    A2 = gn[..., 1] - gn[..., 2] - (b20 - b10)

    Wg2 = np.asarray(p["Wg2"], np.float32)
    w2pad = np.zeros((D, 32), np.float32)
    w2pad[:, 0] = Wg2[:, 1] - Wg2[:, 0]
    w2pad[:, 1] = Wg2[:, 2] - Wg2[:, 0]

    common = {
        "wfuse": p["Wfuse"], "bfuse": p["bfuse"],
        "wg1s": np.ascontiguousarray(p["Wg1"][:D]),
        "wg1t": np.ascontiguousarray(p["Wg1"][D:]),
        "bg1": p["bg1"], "w2pad": w2pad,
        "bf1_g": p["bf1_g"], "bf1_e": p["bf1_e"],
        "wf1_g": p["Wf1_g"], "wf1_e": p["Wf1_e"],
        "wf2_g": p["Wf2_g"], "wf2_e": p["Wf2_e"],
    }
    for nm_d, nm_p in [("wv_g", "Wv_g"), ("wq_g", "Wq_g"), ("wk_g", "Wk_g"),
                       ("wo_g", "Wo_g"), ("wv_e", "Wv_e"), ("wq_e", "Wq_e"),
                       ("wk_e", "Wk_e"), ("wo_e", "Wo_e")]:
        common[nm_d] = p[nm_p]
    common["wq_g"] = np.asarray(common["wq_g"], np.float32) * RSQ_HD
    common["wq_e"] = np.asarray(common["wq_e"], np.float32) * RSQ_HD
    common["bq_g"] = np.asarray(p["bq_g"], np.float32) * RSQ_HD
    common["bq_e"] = np.asarray(p["bq_e"], np.float32) * RSQ_HD
    for nm in ["bv_g", "bq_g", "bk_g", "bo_g", "bv_e", "bq_e", "bk_e", "bo_e",
               "bf2_g", "bf2_e",
               "ln_g1_w", "ln_g1_b", "ln_g2_w", "ln_g2_b",
               "ln_e1_w", "ln_e1_b", "ln_e2_w", "ln_e2_b"]:
        common[nm] = p[nm]

    bf16_names = {"wf1_g", "wf1_e", "wf2_g", "wf2_e", "w2pad"}
    for k in list(common.keys()):
        arr = np.ascontiguousarray(common[k], np.float32)
        if k in bf16_names:
            arr = arr.astype(ml_dtypes.bfloat16)
        common[k] = arr

    in_maps = []
    for c in range(N_CORES):
        b, qc = c // 4, c % 4
        q0 = qc * QN
        m = dict(common)
        m["ge0"] = np.ascontiguousarray(ge[b], np.float32)
        m["ee0"] = np.ascontiguousarray(ee[b], np.float32)
        m["geq0"] = np.ascontiguousarray(ge[b, q0:q0 + QN], np.float32)
        m["eeq0"] = np.ascontiguousarray(ee[b, q0:q0 + QN], np.float32)
        gg = np.stack([A0[:, b, q0:q0 + QN, :], A1[:, b, q0:q0 + QN, :],
                       A2[:, b, q0:q0 + QN, :]], axis=1)      # (2,3,QN,G)
        gg = gg.transpose(0, 1, 3, 2).reshape(L - MASK_START, 3, KC, 128, QN)
        m["gna"] = np.ascontiguousarray(gg, np.float32)
        in_maps.append(m)

    nc = _get_nc()
    trace = os.environ.get("KERNEL_TRACE", "0") == "1"
    if trace:
        import tempfile
        tdir = os.environ.get("KERNEL_TRACE_DIR") or tempfile.mkdtemp(prefix="ktrace_")
        res = run_bass_kernel_spmd(nc, in_maps, core_ids=list(range(N_CORES)),
                                   trace=True, tmpdir=tdir)
        print(f"HW exec time: {res.exec_time_ns} ns")
        print(f"mean exec: {res.mean_exec_time_ns} max core: {res.max_exec_time_core_id}")
        print(f"trace dir: {tdir}")
    else:
        res = run_bass_kernel_spmd(nc, in_maps, core_ids=list(range(N_CORES)))

    gene_out = np.zeros((B, G, D), np.float32)
    expr_out = np.zeros((B, G, D), np.float32)
    for c in range(N_CORES):
        b, qc = c // 4, c % 4
        o = res.results[c]["out"]
        gene_out[b, qc * QN:(qc + 1) * QN] = o[0]
        expr_out[b, qc * QN:(qc + 1) * QN] = o[1]
    return np.stack([gene_out, expr_out]).astype(np.float32)
